# revision 5
# baseline (speedup 1.0000x reference)
"""BatchTopKCrosscoder on 8 Trainium2 NeuronCores.

Strategy (F-sharded tensor parallel, 2 device launches + thin host glue):
  Launch 1 (encode): each core computes actsT_c = relu(W_enc_c^T x^T + b)
      for its F/8 = 4096 dictionary columns, as fp32r (TF32-like) matmuls.
  Host: exact batch-top-(k*B) selection over scores = acts * ||W_dec row||.
      Bulk scores use the device acts; all scores within +-DELTA of the
      boundary are re-derived in float64 from the raw inputs so the chosen
      set matches a full-precision computation. The selection is lowered to
      per-feature activation thresholds thr_f.
  Launch 2 (decode): each core computes sparse_c = acts_c * (acts_c >= thr_c)
      and the partial reconstruction rec_c^T = W_dec_c^T sparse_c^T (+ b_dec
      on core 0 only).
  Host: sum partial reconstructions, apply O(1)-sized mask patches, assemble
      full outputs.
"""
import os
import sys
import types
import contextlib

sys.path.insert(0, "/opt/trn_rl_repo")

import numpy as np

# ---------------------------------------------------------------- constants
B, Z, F, NC = 2048, 1536, 32768, 8
FS = F // NC          # 4096 features per core
BC = 512              # matmul moving free dim
KT = Z // 128         # 12 contraction tiles (encode)
FMT = FS // 128       # 32 feature tiles per core
DELTA = 0.03          # score half-window re-derived in float64
HUGE = np.float32(3e38)

_CACHE = {}


# ------------------------------------------------------- axon profile shim
def _install_profile_shim():
    """Recreate antenv.axon_hooks (absent from the image's antenv stub) so
    run_bass_kernel_spmd(trace=True) can reach the NTFF profiler."""
    if "antenv.axon_hooks" in sys.modules:
        return
    try:
        import antenv
    except ImportError:
        return
    mod = types.ModuleType("antenv.axon_hooks")
    _hook = [None]
    mod.set_axon_ntff_profile_hook = lambda h: _hook.__setitem__(0, h)
    mod.get_axon_ntff_profile_hook = lambda: _hook[0]
    sys.modules["antenv.axon_hooks"] = mod
    antenv.axon_hooks = mod
    try:
        if "/root/.axon_site" not in sys.path:
            sys.path.append("/root/.axon_site")
        from trn_agent_boot.trn_boot import _ntff_profile_via_ctypes
        hook = _ntff_profile_via_ctypes("/opt/axon/libaxon_pjrt.so")
        if hook is not None:
            mod.set_axon_ntff_profile_hook(hook)
    except Exception:
        pass


# ------------------------------------------------------------ NEFF builders
def _env():
    if "env" in _CACHE:
        return _CACHE["env"]
    _install_profile_shim()
    from contextlib import ExitStack
    import concourse.bass as bass
    import concourse.mybir as mybir
    import concourse.tile as tile
    from concourse import bacc
    from concourse.bass_utils import run_bass_kernel_spmd
    _CACHE["env"] = (bass, mybir, tile, bacc, run_bass_kernel_spmd, ExitStack)
    return _CACHE["env"]


def _build_encode():
    bass, mybir, tile, bacc, _, ExitStack = _env()
    F32, F32R = mybir.dt.float32, mybir.dt.float32r
    nc = bacc.Bacc("TRN2", target_bir_lowering=False)
    xT = nc.dram_tensor("xT", [Z, B], F32R, kind="ExternalInput")
    We = nc.dram_tensor("We", [Z, FS], F32R, kind="ExternalInput")
    be = nc.dram_tensor("be", [128, FMT], F32, kind="ExternalInput")
    actsT = nc.dram_tensor("actsT", [FS, B], F32, kind="ExternalOutput")

    with tile.TileContext(nc) as tc, ExitStack() as ctx:
        xpool = ctx.enter_context(tc.tile_pool(name="xp", bufs=1))
        wpool = ctx.enter_context(tc.tile_pool(name="wp", bufs=2))
        opool = ctx.enter_context(tc.tile_pool(name="op", bufs=3))
        cpool = ctx.enter_context(tc.tile_pool(name="cp", bufs=1))
        pspool = ctx.enter_context(tc.tile_pool(name="pp", bufs=2, space="PSUM"))

        b_sb = cpool.tile([128, FMT], F32, name="b_sb")
        nc.sync.dma_start(b_sb[:, :], be[:, :])
        xts = []
        for zc in range(KT):
            t = xpool.tile([128, B], F32R, name=f"xt{zc}")
            nc.sync.dma_start(t[:, :], xT[zc * 128:(zc + 1) * 128, :])
            xts.append(t)

        NG = 512                       # F columns per W load group
        for g in range(FS // NG):      # 8 groups
            wg = wpool.tile([128, KT * NG], F32R, tag="wg", name=f"wg{g}")
            for zc in range(KT):
                nc.scalar.dma_start(
                    wg[:, zc * NG:(zc + 1) * NG],
                    We[zc * 128:(zc + 1) * 128, g * NG:(g + 1) * NG])
            for fm in range(NG // 128):  # 4 feature tiles per group
                ft = g * (NG // 128) + fm
                pss = [pspool.tile([128, BC], F32, tag=f"ps{b}", name=f"ps{ft}_{b}")
                       for b in range(4)]
                for zc in range(KT):
                    lhsT = wg[:, zc * NG + fm * 128: zc * NG + (fm + 1) * 128]
                    for b in range(4):
                        nc.tensor.matmul(
                            pss[b][:, :], lhsT, xts[zc][:, b * BC:(b + 1) * BC],
                            start=(zc == 0), stop=(zc == KT - 1))
                ot = opool.tile([128, B], F32, tag="ot", name=f"ot{ft}")
                for b in range(4):
                    nc.scalar.activation(
                        ot[:, b * BC:(b + 1) * BC], pss[b][:, :],
                        mybir.ActivationFunctionType.Relu, bias=b_sb[:, ft:ft + 1])
                nc.sync.dma_start(actsT[ft * 128:(ft + 1) * 128, :], ot[:, :])
    nc.compile()
    return nc


def _build_decode():
    bass, mybir, tile, bacc, _, ExitStack = _env()
    F32, F32R = mybir.dt.float32, mybir.dt.float32r
    nc = bacc.Bacc("TRN2", target_bir_lowering=False)
    actsT = nc.dram_tensor("actsT", [FS, B], F32, kind="ExternalInput")
    Wd = nc.dram_tensor("Wd", [FS, Z], F32R, kind="ExternalInput")
    thr = nc.dram_tensor("thr", [128, FMT], F32, kind="ExternalInput")
    bdec = nc.dram_tensor("bdec", [128, KT], F32, kind="ExternalInput")
    sparseT = nc.dram_tensor("sparseT", [FS, B], F32, kind="ExternalOutput")
    recT = nc.dram_tensor("recT", [Z, B], F32, kind="ExternalOutput")

    BH = B // 2  # 1024, batch half resident in SBUF
    with tile.TileContext(nc) as tc, ExitStack() as ctx:
        apool = ctx.enter_context(tc.tile_pool(name="ap", bufs=5))
        sppool = ctx.enter_context(tc.tile_pool(name="sp", bufs=1))
        wpool = ctx.enter_context(tc.tile_pool(name="wp", bufs=8))
        rpool = ctx.enter_context(tc.tile_pool(name="rp", bufs=4))
        cpool = ctx.enter_context(tc.tile_pool(name="cp", bufs=1))
        pspool = ctx.enter_context(tc.tile_pool(name="pp", bufs=1, space="PSUM"))

        thr_sb = cpool.tile([128, FMT], F32, name="thr_sb")
        nc.sync.dma_start(thr_sb[:, :], thr[:, :])
        bd_sb = cpool.tile([128, KT], F32, name="bd_sb")
        nc.sync.dma_start(bd_sb[:, :], bdec[:, :])

        for h in range(2):
            sprs = []
            for ft in range(FMT):
                at = apool.tile([128, BH], F32, tag="at", name=f"at{h}_{ft}")
                nc.sync.dma_start(
                    at[:, :], actsT[ft * 128:(ft + 1) * 128, h * BH:(h + 1) * BH])
                sp = sppool.tile([128, BH], F32R, tag=f"sp{ft}", name=f"sp{h}_{ft}")
                nc.vector.scalar_tensor_tensor(
                    sp[:, :], at[:, :], thr_sb[:, ft:ft + 1], at[:, :],
                    op0=mybir.AluOpType.is_ge, op1=mybir.AluOpType.mult)
                nc.gpsimd.dma_start(
                    sparseT[ft * 128:(ft + 1) * 128, h * BH:(h + 1) * BH],
                    sp[:, :].bitcast(F32))
                sprs.append(sp)
            for zg in range(3):  # groups of 4 z-tiles -> 8 psum banks
                ps8 = [pspool.tile([128, BC], F32, tag=f"dp{i}", name=f"dp{h}_{zg}_{i}")
                       for i in range(8)]
                for ft in range(FMT):
                    wt = wpool.tile([128, 512], F32R, tag="wt", name=f"wt{h}_{zg}_{ft}")
                    nc.scalar.dma_start(
                        wt[:, :], Wd[ft * 128:(ft + 1) * 128, zg * 512:(zg + 1) * 512])
                    for zi in range(4):
                        lhsT = wt[:, zi * 128:(zi + 1) * 128]
                        for s in range(2):
                            nc.tensor.matmul(
                                ps8[zi * 2 + s][:, :], lhsT,
                                sprs[ft][:, s * BC:(s + 1) * BC],
                                start=(ft == 0), stop=(ft == FMT - 1))
                for zi in range(4):
                    zcol = zg * 4 + zi
                    for s in range(2):
                        rt = rpool.tile([128, BC], F32, tag="rt",
                                        name=f"rt{h}_{zg}_{zi}_{s}")
                        nc.vector.tensor_scalar_add(
                            rt[:, :], ps8[zi * 2 + s][:, :], bd_sb[:, zcol:zcol + 1])
                        nc.gpsimd.dma_start(
                            recT[zcol * 128:(zcol + 1) * 128,
                                 h * BH + s * BC: h * BH + (s + 1) * BC],
                            rt[:, :])
    nc.compile()
    return nc


def _get_neffs():
    if "enc" not in _CACHE:
        _CACHE["enc"] = _build_encode()
        _CACHE["dec"] = _build_decode()
    return _CACHE["enc"], _CACHE["dec"]


def _run(nc, in_maps):
    _, _, _, _, run_bass_kernel_spmd, _ = _env()
    tr = os.environ.get("KERNEL_TRACE", "")
    kw = {}
    if tr:
        cores = list(range(NC)) if tr == "all" else [0]
        kw = dict(trace=True, trace_cores=cores)
    trace = bool(tr)
    try:
        res = run_bass_kernel_spmd(nc, in_maps, core_ids=list(range(NC)), **kw)
    except Exception:
        if not kw:
            raise
        # tracing occasionally races the first execute; retry untraced
        res = run_bass_kernel_spmd(nc, in_maps, core_ids=list(range(NC)))
    if kw and res.exec_time_ns is not None:
        _CACHE.setdefault("exec_times", []).append(res.exec_time_ns)
    return res.results


# ------------------------------------------------------------- host helpers
def _kth_largest(flat, kB):
    """Exact kB-th largest of a 1-D float32 array (prefilter + partition)."""
    n = flat.size
    stride = max(1, n // 200000)
    sample = flat[::stride]
    m = sample.size
    want = max(1, min(m - 1, int(kB / n * m * 1.6) + 8))
    t0 = np.partition(sample, m - want)[m - want]
    cand = flat[flat >= t0]
    if cand.size < kB:  # prefilter too aggressive; fall back
        cand = flat
    return np.partition(cand, cand.size - kB)[cand.size - kB]


def _rne_fp32r(a):
    """Round-to-nearest-even to 11 mantissa bits (device fp32r rounding)."""
    u = a.view(np.uint32).astype(np.uint64)
    half = np.uint64(1 << 11)
    lsb = (u >> np.uint64(12)) & np.uint64(1)
    u2 = ((u + (half - np.uint64(1)) + lsb) >> np.uint64(12)) << np.uint64(12)
    return (u2 & np.uint64(0xFFFFFFFF)).astype(np.uint32).view(np.float32)


# ------------------------------------------------------------------- kernel
def kernel(x_BZ, W_encoder_ZF, b_encoder_F, W_decoder_FZ, b_decoder_Z, k):
    k = int(np.asarray(k))
    kB = k * B
    x = np.ascontiguousarray(np.asarray(x_BZ, dtype=np.float32))
    We = np.asarray(W_encoder_ZF)
    if We.dtype != np.float32:
        We = We.astype(np.float32)
    Wd = np.ascontiguousarray(np.asarray(W_decoder_FZ, dtype=np.float32))
    be = np.ascontiguousarray(np.asarray(b_encoder_F, dtype=np.float32))
    bd = np.ascontiguousarray(np.asarray(b_decoder_Z, dtype=np.float32))

    nc_enc, nc_dec = _get_neffs()

    # ---------------- launch 1: encode
    xT = np.ascontiguousarray(x.T)
    enc_maps = []
    for c in range(NC):
        We_c = np.ascontiguousarray(We[:, c * FS:(c + 1) * FS])
        be_c = np.ascontiguousarray(be[c * FS:(c + 1) * FS].reshape(FMT, 128).T)
        enc_maps.append(dict(xT=xT, We=We_c, be=be_c))
    res1 = _run(nc_enc, enc_maps)
    actsT_sh = [r["actsT"] for r in res1]  # NC x [FS, B] float32

    # ---------------- host: exact top-(k*B) selection
    Wd64 = Wd.astype(np.float64)
    norms64 = np.sqrt(np.einsum("ij,ij->i", Wd64, Wd64))
    norms32 = norms64.astype(np.float32)

    scores = np.empty((F, B), np.float32)
    for c in range(NC):
        np.multiply(actsT_sh[c], norms32[c * FS:(c + 1) * FS, None],
                    out=scores[c * FS:(c + 1) * FS])
    flat = scores.ravel()
    vb = float(_kth_largest(flat, kB))
    hi, lo = vb + DELTA, vb - DELTA

    above = flat > hi
    n_above = int(above.sum())
    cand_idx = np.flatnonzero((flat >= lo) & (flat <= hi))
    need = kB - n_above
    if not (0 < need <= cand_idx.size):
        # window missed (shouldn't happen); widen to brute force
        cand_idx = np.flatnonzero(flat >= lo)
        need = kB - 0
        n_above = 0
        above = np.zeros_like(flat, dtype=bool)

    f_idx = (cand_idx // B).astype(np.int64)
    b_idx = (cand_idx % B).astype(np.int64)
    x64 = x.astype(np.float64)
    WeT = We.T  # [F, Z] view
    g64 = (x64[b_idx] * WeT[f_idx].astype(np.float64)).sum(axis=1)
    acts_cand64 = np.maximum(g64 + be[f_idx].astype(np.float64), 0.0)
    s_cand64 = acts_cand64 * norms64[f_idx]
    order = np.argsort(-s_cand64, kind="stable")
    sel_cand = cand_idx[order[:need]]

    # final mask = above  |  sel_cand ; lower to per-feature act thresholds
    acts_flat_dev = np.empty((F, B), np.float32)
    for c in range(NC):
        acts_flat_dev[c * FS:(c + 1) * FS] = actsT_sh[c]
    thr_f = np.full(F, HUGE, np.float32)
    above2d = above.reshape(F, B)
    anyrow = above2d.any(axis=1)
    masked = np.where(above2d[anyrow], acts_flat_dev[anyrow], HUGE)
    thr_f[anyrow] = masked.min(axis=1)
    if sel_cand.size:
        fsel = (sel_cand // B).astype(np.int64)
        vals = acts_flat_dev.ravel()[sel_cand]
        np.minimum.at(thr_f, fsel, vals)

    # device mask that thr_f realizes, and its diff vs the final mask
    dev_mask = acts_flat_dev >= thr_f[:, None]
    final_lin = np.concatenate([np.flatnonzero(above), sel_cand])
    final_mask = np.zeros(F * B, bool)
    final_mask[final_lin] = True
    final_mask = final_mask.reshape(F, B)
    diff_add = np.flatnonzero(final_mask.ravel() & ~dev_mask.ravel())   # missing on device
    diff_del = np.flatnonzero(~final_mask.ravel() & dev_mask.ravel())   # extra on device

    # ---------------- launch 2: decode
    zeros_bd = np.zeros((128, KT), np.float32)
    bd_arr = np.ascontiguousarray(bd.reshape(KT, 128).T)
    dec_maps = []
    for c in range(NC):
        thr_c = np.ascontiguousarray(
            thr_f[c * FS:(c + 1) * FS].reshape(FMT, 128).T)
        dec_maps.append(dict(
            actsT=actsT_sh[c],
            Wd=Wd[c * FS:(c + 1) * FS],
            thr=thr_c,
            bdec=bd_arr if c == 0 else zeros_bd,
        ))
    res2 = _run(nc_dec, dec_maps)

    # ---------------- host: assemble + patch
    sparse_full = np.concatenate([r["sparseT"] for r in res2], axis=0)  # [F, B]
    rec64 = np.zeros((Z, B), np.float64)
    for r in res2:
        rec64 += r["recT"].astype(np.float64)

    sp_flat = sparse_full.ravel()
    for lin in diff_add:
        f, b = int(lin // B), int(lin % B)
        a = acts_flat_dev[f, b]
        sp_flat[lin] = a
        rec64[:, b] += float(a) * Wd64[f]
    for lin in diff_del:
        f, b = int(lin // B), int(lin % B)
        a = sp_flat[lin]
        sp_flat[lin] = 0.0
        rec64[:, b] -= float(a) * Wd64[f]

    recon = np.ascontiguousarray(rec64.T.astype(np.float32))
    return recon, sparse_full.T, acts_flat_dev.T


# revision 7
# speedup vs baseline: 1.0868x; 1.0868x over previous
"""BatchTopKCrosscoder on 8 Trainium2 NeuronCores.

Strategy (F-sharded tensor parallel, 2 device launches + thin host glue):
  Launch 1 (encode): each core computes actsT_c = relu(W_enc_c^T x^T + b)
      for its F/8 = 4096 dictionary columns, as fp32r (TF32-like) matmuls.
  Host: exact batch-top-(k*B) selection over scores = acts * ||W_dec row||.
      Bulk scores use the device acts; all scores within +-DELTA of the
      boundary are re-derived in float64 from the raw inputs so the chosen
      set matches a full-precision computation. The selection is lowered to
      per-feature activation thresholds thr_f.
  Launch 2 (decode): each core computes sparse_c = acts_c * (acts_c >= thr_c)
      and the partial reconstruction rec_c^T = W_dec_c^T sparse_c^T (+ b_dec
      on core 0 only).
  Host: sum partial reconstructions, apply O(1)-sized mask patches, assemble
      full outputs.
"""
import os
import sys
import types
import contextlib

sys.path.insert(0, "/opt/trn_rl_repo")

import numpy as np

# ---------------------------------------------------------------- constants
B, Z, F, NC = 2048, 1536, 32768, 8
FS = F // NC          # 4096 features per core
BC = 512              # matmul moving free dim
KT = Z // 128         # 12 contraction tiles (encode)
FMT = FS // 128       # 32 feature tiles per core
DELTA = 0.03          # score half-window re-derived in float64
HUGE = np.float32(3e38)

_CACHE = {}


# ------------------------------------------------------- axon profile shim
def _install_profile_shim():
    """Recreate antenv.axon_hooks (absent from the image's antenv stub) so
    run_bass_kernel_spmd(trace=True) can reach the NTFF profiler."""
    if "antenv.axon_hooks" in sys.modules:
        return
    try:
        import antenv
    except ImportError:
        return
    mod = types.ModuleType("antenv.axon_hooks")
    _hook = [None]
    mod.set_axon_ntff_profile_hook = lambda h: _hook.__setitem__(0, h)
    mod.get_axon_ntff_profile_hook = lambda: _hook[0]
    sys.modules["antenv.axon_hooks"] = mod
    antenv.axon_hooks = mod
    try:
        if "/root/.axon_site" not in sys.path:
            sys.path.append("/root/.axon_site")
        from trn_agent_boot.trn_boot import _ntff_profile_via_ctypes
        hook = _ntff_profile_via_ctypes("/opt/axon/libaxon_pjrt.so")
        if hook is not None:
            mod.set_axon_ntff_profile_hook(hook)
    except Exception:
        pass


# ------------------------------------------------------------ NEFF builders
def _env():
    if "env" in _CACHE:
        return _CACHE["env"]
    _install_profile_shim()
    from contextlib import ExitStack
    import concourse.bass as bass
    import concourse.mybir as mybir
    import concourse.tile as tile
    from concourse import bacc
    from concourse.bass_utils import run_bass_kernel_spmd
    _CACHE["env"] = (bass, mybir, tile, bacc, run_bass_kernel_spmd, ExitStack)
    return _CACHE["env"]


def _build_encode():
    bass, mybir, tile, bacc, _, ExitStack = _env()
    F32, F32R = mybir.dt.float32, mybir.dt.float32r
    nc = bacc.Bacc("TRN2", target_bir_lowering=False)
    xT = nc.dram_tensor("xT", [Z, B], F32R, kind="ExternalInput")
    We = nc.dram_tensor("We", [Z, FS], F32R, kind="ExternalInput")
    be = nc.dram_tensor("be", [128, FMT], F32, kind="ExternalInput")
    actsT = nc.dram_tensor("actsT", [FS, B], F32, kind="ExternalOutput")

    with tile.TileContext(nc) as tc, ExitStack() as ctx:
        xpool = ctx.enter_context(tc.tile_pool(name="xp", bufs=1))
        wpool = ctx.enter_context(tc.tile_pool(name="wp", bufs=2))
        opool = ctx.enter_context(tc.tile_pool(name="op", bufs=3))
        cpool = ctx.enter_context(tc.tile_pool(name="cp", bufs=1))
        pspool = ctx.enter_context(tc.tile_pool(name="pp", bufs=2, space="PSUM"))

        b_sb = cpool.tile([128, FMT], F32, name="b_sb")
        nc.sync.dma_start(b_sb[:, :], be[:, :])
        xts = []
        for zc in range(KT):
            t = xpool.tile([128, B], F32R, name=f"xt{zc}")
            nc.sync.dma_start(t[:, :], xT[zc * 128:(zc + 1) * 128, :])
            xts.append(t)

        NG = 512                       # F columns per W load group
        for g in range(FS // NG):      # 8 groups
            wg = wpool.tile([128, KT * NG], F32R, tag="wg", name=f"wg{g}")
            for zc in range(KT):
                nc.scalar.dma_start(
                    wg[:, zc * NG:(zc + 1) * NG],
                    We[zc * 128:(zc + 1) * 128, g * NG:(g + 1) * NG])
            for fm in range(NG // 128):  # 4 feature tiles per group
                ft = g * (NG // 128) + fm
                pss = [pspool.tile([128, BC], F32, tag=f"ps{b}", name=f"ps{ft}_{b}")
                       for b in range(4)]
                for zc in range(KT):
                    lhsT = wg[:, zc * NG + fm * 128: zc * NG + (fm + 1) * 128]
                    for b in range(4):
                        nc.tensor.matmul(
                            pss[b][:, :], lhsT, xts[zc][:, b * BC:(b + 1) * BC],
                            start=(zc == 0), stop=(zc == KT - 1))
                ot = opool.tile([128, B], F32, tag="ot", name=f"ot{ft}")
                for b in range(4):
                    nc.scalar.activation(
                        ot[:, b * BC:(b + 1) * BC], pss[b][:, :],
                        mybir.ActivationFunctionType.Relu, bias=b_sb[:, ft:ft + 1])
                nc.sync.dma_start(actsT[ft * 128:(ft + 1) * 128, :], ot[:, :])
    nc.compile()
    return nc


def _build_decode():
    bass, mybir, tile, bacc, _, ExitStack = _env()
    F32, BF16 = mybir.dt.float32, mybir.dt.bfloat16
    nc = bacc.Bacc("TRN2", target_bir_lowering=False)
    actsT = nc.dram_tensor("actsT", [FS, B], F32, kind="ExternalInput")
    Wd = nc.dram_tensor("Wd", [FS, Z], BF16, kind="ExternalInput")
    thr = nc.dram_tensor("thr", [128, FMT], F32, kind="ExternalInput")
    bdec = nc.dram_tensor("bdec", [128, KT], F32, kind="ExternalInput")
    sparseT = nc.dram_tensor("sparseT", [FS, B], BF16, kind="ExternalOutput")
    recT = nc.dram_tensor("recT", [Z, B], F32, kind="ExternalOutput")

    NB = 512  # moving free dim (PSUM bank limit)
    with tile.TileContext(nc) as tc, ExitStack() as ctx:
        apool = ctx.enter_context(tc.tile_pool(name="ap", bufs=5))
        sppool = ctx.enter_context(tc.tile_pool(name="sp", bufs=1))
        wpool = ctx.enter_context(tc.tile_pool(name="wp", bufs=8))
        rpool = ctx.enter_context(tc.tile_pool(name="rp", bufs=4))
        cpool = ctx.enter_context(tc.tile_pool(name="cp", bufs=1))
        pspool = ctx.enter_context(tc.tile_pool(name="pp", bufs=2, space="PSUM"))

        thr_sb = cpool.tile([128, FMT], F32, name="thr_sb")
        nc.sync.dma_start(thr_sb[:, :], thr[:, :])
        bd_sb = cpool.tile([128, KT], F32, name="bd_sb")
        nc.sync.dma_start(bd_sb[:, :], bdec[:, :])

        # sparse production: full batch resident in bf16 (16 MB)
        sprs = []
        for ft in range(FMT):
            at = apool.tile([128, B], F32, tag="at", name=f"at{ft}")
            nc.sync.dma_start(at[:, :], actsT[ft * 128:(ft + 1) * 128, :])
            sp = sppool.tile([128, B], BF16, name=f"sp{ft}")
            nc.vector.scalar_tensor_tensor(
                sp[:, :], at[:, :], thr_sb[:, ft:ft + 1], at[:, :],
                op0=mybir.AluOpType.is_ge, op1=mybir.AluOpType.mult)
            nc.gpsimd.dma_start(
                sparseT[ft * 128:(ft + 1) * 128, :], sp[:, :])
            sprs.append(sp)

        for zg in range(KT):  # 12 groups of 128 z-cols; 4 psum tiles x2 bufs
            ps4 = [pspool.tile([128, NB], F32, tag=f"dp{s}", name=f"dp{zg}_{s}")
                   for s in range(4)]
            for ft in range(FMT):
                wt = wpool.tile([128, 128], BF16, tag="wt", name=f"wt{zg}_{ft}")
                nc.scalar.dma_start(
                    wt[:, :], Wd[ft * 128:(ft + 1) * 128, zg * 128:(zg + 1) * 128])
                for s in range(4):
                    nc.tensor.matmul(
                        ps4[s][:, :], wt[:, :], sprs[ft][:, s * NB:(s + 1) * NB],
                        start=(ft == 0), stop=(ft == FMT - 1))
            for s in range(4):
                rt = rpool.tile([128, NB], F32, tag="rt", name=f"rt{zg}_{s}")
                nc.vector.tensor_scalar_add(
                    rt[:, :], ps4[s][:, :], bd_sb[:, zg:zg + 1])
                nc.gpsimd.dma_start(
                    recT[zg * 128:(zg + 1) * 128, s * NB:(s + 1) * NB], rt[:, :])
    nc.compile()
    return nc


def _get_neffs():
    if "enc" not in _CACHE:
        _CACHE["enc"] = _build_encode()
        _CACHE["dec"] = _build_decode()
    return _CACHE["enc"], _CACHE["dec"]


def _run(nc, in_maps):
    _, _, _, _, run_bass_kernel_spmd, _ = _env()
    tr = os.environ.get("KERNEL_TRACE", "")
    kw = {}
    if tr:
        cores = list(range(NC)) if tr == "all" else [0]
        kw = dict(trace=True, trace_cores=cores)
    trace = bool(tr)
    try:
        res = run_bass_kernel_spmd(nc, in_maps, core_ids=list(range(NC)), **kw)
    except Exception:
        if not kw:
            raise
        # tracing occasionally races the first execute; retry untraced
        res = run_bass_kernel_spmd(nc, in_maps, core_ids=list(range(NC)))
    if kw and res.exec_time_ns is not None:
        _CACHE.setdefault("exec_times", []).append(res.exec_time_ns)
    return res.results


# ------------------------------------------------------------- host helpers
def _kth_largest(flat, kB):
    """Exact kB-th largest of a 1-D float32 array (prefilter + partition)."""
    n = flat.size
    stride = max(1, n // 200000)
    sample = flat[::stride]
    m = sample.size
    want = max(1, min(m - 1, int(kB / n * m * 1.6) + 8))
    t0 = np.partition(sample, m - want)[m - want]
    cand = flat[flat >= t0]
    if cand.size < kB:  # prefilter too aggressive; fall back
        cand = flat
    return np.partition(cand, cand.size - kB)[cand.size - kB]


def _rne_fp32r(a):
    """Round-to-nearest-even to 11 mantissa bits (device fp32r rounding)."""
    u = a.view(np.uint32).astype(np.uint64)
    half = np.uint64(1 << 11)
    lsb = (u >> np.uint64(12)) & np.uint64(1)
    u2 = ((u + (half - np.uint64(1)) + lsb) >> np.uint64(12)) << np.uint64(12)
    return (u2 & np.uint64(0xFFFFFFFF)).astype(np.uint32).view(np.float32)


# ------------------------------------------------------------------- kernel
def kernel(x_BZ, W_encoder_ZF, b_encoder_F, W_decoder_FZ, b_decoder_Z, k):
    k = int(np.asarray(k))
    kB = k * B
    x = np.ascontiguousarray(np.asarray(x_BZ, dtype=np.float32))
    We = np.asarray(W_encoder_ZF)
    if We.dtype != np.float32:
        We = We.astype(np.float32)
    Wd = np.ascontiguousarray(np.asarray(W_decoder_FZ, dtype=np.float32))
    be = np.ascontiguousarray(np.asarray(b_encoder_F, dtype=np.float32))
    bd = np.ascontiguousarray(np.asarray(b_decoder_Z, dtype=np.float32))

    nc_enc, nc_dec = _get_neffs()

    # ---------------- launch 1: encode
    xT = np.ascontiguousarray(x.T)
    enc_maps = []
    for c in range(NC):
        We_c = np.ascontiguousarray(We[:, c * FS:(c + 1) * FS])
        be_c = np.ascontiguousarray(be[c * FS:(c + 1) * FS].reshape(FMT, 128).T)
        enc_maps.append(dict(xT=xT, We=We_c, be=be_c))
    res1 = _run(nc_enc, enc_maps)
    actsT_sh = [r["actsT"] for r in res1]  # NC x [FS, B] float32

    # ---------------- host: exact top-(k*B) selection
    Wd64 = Wd.astype(np.float64)
    norms64 = np.sqrt(np.einsum("ij,ij->i", Wd64, Wd64))
    norms32 = norms64.astype(np.float32)

    scores = np.empty((F, B), np.float32)
    for c in range(NC):
        np.multiply(actsT_sh[c], norms32[c * FS:(c + 1) * FS, None],
                    out=scores[c * FS:(c + 1) * FS])
    flat = scores.ravel()
    vb = float(_kth_largest(flat, kB))
    hi, lo = vb + DELTA, vb - DELTA

    above = flat > hi
    n_above = int(above.sum())
    cand_idx = np.flatnonzero((flat >= lo) & (flat <= hi))
    need = kB - n_above
    if not (0 < need <= cand_idx.size):
        # window missed (shouldn't happen); widen to brute force
        cand_idx = np.flatnonzero(flat >= lo)
        need = kB - 0
        n_above = 0
        above = np.zeros_like(flat, dtype=bool)

    f_idx = (cand_idx // B).astype(np.int64)
    b_idx = (cand_idx % B).astype(np.int64)
    x64 = x.astype(np.float64)
    WeT = We.T  # [F, Z] view
    g64 = (x64[b_idx] * WeT[f_idx].astype(np.float64)).sum(axis=1)
    acts_cand64 = np.maximum(g64 + be[f_idx].astype(np.float64), 0.0)
    s_cand64 = acts_cand64 * norms64[f_idx]
    order = np.argsort(-s_cand64, kind="stable")
    sel_cand = cand_idx[order[:need]]

    # final mask = above  |  sel_cand ; lower to per-feature act thresholds
    acts_flat_dev = np.empty((F, B), np.float32)
    for c in range(NC):
        acts_flat_dev[c * FS:(c + 1) * FS] = actsT_sh[c]
    thr_f = np.full(F, HUGE, np.float32)
    above2d = above.reshape(F, B)
    anyrow = above2d.any(axis=1)
    masked = np.where(above2d[anyrow], acts_flat_dev[anyrow], HUGE)
    thr_f[anyrow] = masked.min(axis=1)
    if sel_cand.size:
        fsel = (sel_cand // B).astype(np.int64)
        vals = acts_flat_dev.ravel()[sel_cand]
        np.minimum.at(thr_f, fsel, vals)

    # device mask that thr_f realizes, and its diff vs the final mask
    dev_mask = acts_flat_dev >= thr_f[:, None]
    final_lin = np.concatenate([np.flatnonzero(above), sel_cand])
    final_mask = np.zeros(F * B, bool)
    final_mask[final_lin] = True
    final_mask = final_mask.reshape(F, B)
    diff_add = np.flatnonzero(final_mask.ravel() & ~dev_mask.ravel())   # missing on device
    diff_del = np.flatnonzero(~final_mask.ravel() & dev_mask.ravel())   # extra on device

    # ---------------- launch 2: decode
    import ml_dtypes
    Wd_bf16 = Wd.astype(ml_dtypes.bfloat16)
    zeros_bd = np.zeros((128, KT), np.float32)
    bd_arr = np.ascontiguousarray(bd.reshape(KT, 128).T)
    dec_maps = []
    for c in range(NC):
        thr_c = np.ascontiguousarray(
            thr_f[c * FS:(c + 1) * FS].reshape(FMT, 128).T)
        dec_maps.append(dict(
            actsT=actsT_sh[c],
            Wd=Wd_bf16[c * FS:(c + 1) * FS],
            thr=thr_c,
            bdec=bd_arr if c == 0 else zeros_bd,
        ))
    res2 = _run(nc_dec, dec_maps)

    # ---------------- host: assemble + patch
    sparse_full = np.concatenate(
        [r["sparseT"].astype(np.float32) for r in res2], axis=0)  # [F, B]
    rec64 = np.zeros((Z, B), np.float64)
    for r in res2:
        rec64 += r["recT"].astype(np.float64)

    sp_flat = sparse_full.ravel()
    for lin in diff_add:
        f, b = int(lin // B), int(lin % B)
        a = acts_flat_dev[f, b]
        sp_flat[lin] = a
        rec64[:, b] += float(a) * Wd64[f]
    for lin in diff_del:
        f, b = int(lin // B), int(lin % B)
        a = sp_flat[lin]
        sp_flat[lin] = 0.0
        rec64[:, b] -= float(a) * Wd64[f]

    recon = np.ascontiguousarray(rec64.T.astype(np.float32))
    return recon, sparse_full.T, acts_flat_dev.T


# revision 8
# speedup vs baseline: 1.1006x; 1.0127x over previous
"""BatchTopKCrosscoder on 8 Trainium2 NeuronCores.

Strategy (F-sharded tensor parallel, 2 device launches + thin host glue):
  Launch 1 (encode): each core computes actsT_c = relu(W_enc_c^T x^T + b)
      for its F/8 = 4096 dictionary columns, as fp32r (TF32-like) matmuls.
  Host: exact batch-top-(k*B) selection over scores = acts * ||W_dec row||.
      Bulk scores use the device acts; all scores within +-DELTA of the
      boundary are re-derived in float64 from the raw inputs so the chosen
      set matches a full-precision computation. The selection is lowered to
      per-feature activation thresholds thr_f.
  Launch 2 (decode): each core computes sparse_c = acts_c * (acts_c >= thr_c)
      and the partial reconstruction rec_c^T = W_dec_c^T sparse_c^T (+ b_dec
      on core 0 only).
  Host: sum partial reconstructions, apply O(1)-sized mask patches, assemble
      full outputs.
"""
import os
import sys
import types
import contextlib

sys.path.insert(0, "/opt/trn_rl_repo")

import numpy as np

# ---------------------------------------------------------------- constants
B, Z, F, NC = 2048, 1536, 32768, 8
FS = F // NC          # 4096 features per core
BC = 512              # matmul moving free dim
KT = Z // 128         # 12 contraction tiles (encode)
FMT = FS // 128       # 32 feature tiles per core
DELTA = 0.03          # score half-window re-derived in float64
HUGE = np.float32(3e38)

_CACHE = {}


# ------------------------------------------------------- axon profile shim
def _install_profile_shim():
    """Recreate antenv.axon_hooks (absent from the image's antenv stub) so
    run_bass_kernel_spmd(trace=True) can reach the NTFF profiler."""
    if "antenv.axon_hooks" in sys.modules:
        return
    try:
        import antenv
    except ImportError:
        return
    mod = types.ModuleType("antenv.axon_hooks")
    _hook = [None]
    mod.set_axon_ntff_profile_hook = lambda h: _hook.__setitem__(0, h)
    mod.get_axon_ntff_profile_hook = lambda: _hook[0]
    sys.modules["antenv.axon_hooks"] = mod
    antenv.axon_hooks = mod
    try:
        if "/root/.axon_site" not in sys.path:
            sys.path.append("/root/.axon_site")
        from trn_agent_boot.trn_boot import _ntff_profile_via_ctypes
        hook = _ntff_profile_via_ctypes("/opt/axon/libaxon_pjrt.so")
        if hook is not None:
            mod.set_axon_ntff_profile_hook(hook)
    except Exception:
        pass


# ------------------------------------------------------------ NEFF builders
def _env():
    if "env" in _CACHE:
        return _CACHE["env"]
    _install_profile_shim()
    from contextlib import ExitStack
    import concourse.bass as bass
    import concourse.mybir as mybir
    import concourse.tile as tile
    from concourse import bacc
    from concourse.bass_utils import run_bass_kernel_spmd
    _CACHE["env"] = (bass, mybir, tile, bacc, run_bass_kernel_spmd, ExitStack)
    return _CACHE["env"]


def _build_encode():
    bass, mybir, tile, bacc, _, ExitStack = _env()
    F32, F32R = mybir.dt.float32, mybir.dt.float32r
    nc = bacc.Bacc("TRN2", target_bir_lowering=False)
    xT = nc.dram_tensor("xT", [Z, B], F32R, kind="ExternalInput")
    We = nc.dram_tensor("We", [Z, FS], F32R, kind="ExternalInput")
    be = nc.dram_tensor("be", [128, FMT], F32, kind="ExternalInput")
    actsT = nc.dram_tensor("actsT", [FS, B], F32, kind="ExternalOutput")

    with tile.TileContext(nc) as tc, ExitStack() as ctx:
        xpool = ctx.enter_context(tc.tile_pool(name="xp", bufs=1))
        wpool = ctx.enter_context(tc.tile_pool(name="wp", bufs=2))
        opool = ctx.enter_context(tc.tile_pool(name="op", bufs=3))
        cpool = ctx.enter_context(tc.tile_pool(name="cp", bufs=1))
        pspool = ctx.enter_context(tc.tile_pool(name="pp", bufs=2, space="PSUM"))

        b_sb = cpool.tile([128, FMT], F32, name="b_sb")
        nc.sync.dma_start(b_sb[:, :], be[:, :])
        xts = []
        for zc in range(KT):
            t = xpool.tile([128, B], F32R, name=f"xt{zc}")
            nc.sync.dma_start(t[:, :], xT[zc * 128:(zc + 1) * 128, :])
            xts.append(t)

        NG = 512                       # F columns per W load group
        for g in range(FS // NG):      # 8 groups
            wg = wpool.tile([128, KT * NG], F32R, tag="wg", name=f"wg{g}")
            for zc in range(KT):
                nc.scalar.dma_start(
                    wg[:, zc * NG:(zc + 1) * NG],
                    We[zc * 128:(zc + 1) * 128, g * NG:(g + 1) * NG])
            for fm in range(NG // 128):  # 4 feature tiles per group
                ft = g * (NG // 128) + fm
                pss = [pspool.tile([128, BC], F32, tag=f"ps{b}", name=f"ps{ft}_{b}")
                       for b in range(4)]
                for zc in range(KT):
                    lhsT = wg[:, zc * NG + fm * 128: zc * NG + (fm + 1) * 128]
                    for b in range(4):
                        nc.tensor.matmul(
                            pss[b][:, :], lhsT, xts[zc][:, b * BC:(b + 1) * BC],
                            start=(zc == 0), stop=(zc == KT - 1))
                ot = opool.tile([128, B], F32, tag="ot", name=f"ot{ft}")
                for b in range(4):
                    nc.scalar.activation(
                        ot[:, b * BC:(b + 1) * BC], pss[b][:, :],
                        mybir.ActivationFunctionType.Relu, bias=b_sb[:, ft:ft + 1])
                nc.sync.dma_start(actsT[ft * 128:(ft + 1) * 128, :], ot[:, :])
    nc.compile()
    return nc


def _build_decode():
    bass, mybir, tile, bacc, _, ExitStack = _env()
    F32, BF16 = mybir.dt.float32, mybir.dt.bfloat16
    nc = bacc.Bacc("TRN2", target_bir_lowering=False)
    actsT = nc.dram_tensor("actsT", [FS, B], F32, kind="ExternalInput")
    Wd = nc.dram_tensor("Wd", [FS, Z], BF16, kind="ExternalInput")
    thr = nc.dram_tensor("thr", [128, FMT], F32, kind="ExternalInput")
    bdec = nc.dram_tensor("bdec", [128, KT], F32, kind="ExternalInput")
    sparseT = nc.dram_tensor("sparseT", [FS, B], BF16, kind="ExternalOutput")
    recT = nc.dram_tensor("recT", [Z, B], F32, kind="ExternalOutput")

    NB = 512  # moving free dim (PSUM bank limit)
    with tile.TileContext(nc) as tc, ExitStack() as ctx:
        apool = ctx.enter_context(tc.tile_pool(name="ap", bufs=5))
        sppool = ctx.enter_context(tc.tile_pool(name="sp", bufs=1))
        wpool = ctx.enter_context(tc.tile_pool(name="wp", bufs=8))
        rpool = ctx.enter_context(tc.tile_pool(name="rp", bufs=4))
        cpool = ctx.enter_context(tc.tile_pool(name="cp", bufs=1))
        pspool = ctx.enter_context(tc.tile_pool(name="pp", bufs=2, space="PSUM"))

        thr_sb = cpool.tile([128, FMT], F32, name="thr_sb")
        nc.sync.dma_start(thr_sb[:, :], thr[:, :])
        bd_sb = cpool.tile([128, KT], F32, name="bd_sb")
        nc.sync.dma_start(bd_sb[:, :], bdec[:, :])

        # sparse production: full batch resident in bf16 (16 MB)
        sprs = []
        for ft in range(FMT):
            at = apool.tile([128, B], F32, tag="at", name=f"at{ft}")
            eng = nc.sync if ft % 2 == 0 else nc.scalar
            eng.dma_start(at[:, :], actsT[ft * 128:(ft + 1) * 128, :])
            sp = sppool.tile([128, B], BF16, name=f"sp{ft}")
            nc.vector.scalar_tensor_tensor(
                sp[:, :], at[:, :], thr_sb[:, ft:ft + 1], at[:, :],
                op0=mybir.AluOpType.is_ge, op1=mybir.AluOpType.mult)
            nc.gpsimd.dma_start(
                sparseT[ft * 128:(ft + 1) * 128, :], sp[:, :])
            sprs.append(sp)

        for zg in range(KT):  # 12 groups of 128 z-cols; 4 psum tiles x2 bufs
            ps4 = [pspool.tile([128, NB], F32, tag=f"dp{s}", name=f"dp{zg}_{s}")
                   for s in range(4)]
            for ft in range(FMT):
                wt = wpool.tile([128, 128], BF16, tag="wt", name=f"wt{zg}_{ft}")
                nc.scalar.dma_start(
                    wt[:, :], Wd[ft * 128:(ft + 1) * 128, zg * 128:(zg + 1) * 128])
                for s in range(4):
                    nc.tensor.matmul(
                        ps4[s][:, :], wt[:, :], sprs[ft][:, s * NB:(s + 1) * NB],
                        start=(ft == 0), stop=(ft == FMT - 1))
            for s in range(4):
                rt = rpool.tile([128, NB], F32, tag="rt", name=f"rt{zg}_{s}")
                nc.vector.tensor_scalar_add(
                    rt[:, :], ps4[s][:, :], bd_sb[:, zg:zg + 1])
                nc.gpsimd.dma_start(
                    recT[zg * 128:(zg + 1) * 128, s * NB:(s + 1) * NB], rt[:, :])
    nc.compile()
    return nc


def _get_neffs():
    if "enc" not in _CACHE:
        _CACHE["enc"] = _build_encode()
        _CACHE["dec"] = _build_decode()
    return _CACHE["enc"], _CACHE["dec"]


def _run(nc, in_maps):
    _, _, _, _, run_bass_kernel_spmd, _ = _env()
    tr = os.environ.get("KERNEL_TRACE", "")
    kw = {}
    if tr:
        cores = list(range(NC)) if tr == "all" else [0]
        kw = dict(trace=True, trace_cores=cores)
    trace = bool(tr)
    try:
        res = run_bass_kernel_spmd(nc, in_maps, core_ids=list(range(NC)), **kw)
    except Exception:
        if not kw:
            raise
        # tracing occasionally races the first execute; retry untraced
        res = run_bass_kernel_spmd(nc, in_maps, core_ids=list(range(NC)))
    if kw and res.exec_time_ns is not None:
        _CACHE.setdefault("exec_times", []).append(res.exec_time_ns)
    return res.results


# ------------------------------------------------------------- host helpers
def _kth_largest(flat, kB):
    """Exact kB-th largest of a 1-D float32 array (prefilter + partition)."""
    n = flat.size
    stride = max(1, n // 200000)
    sample = flat[::stride]
    m = sample.size
    want = max(1, min(m - 1, int(kB / n * m * 1.6) + 8))
    t0 = np.partition(sample, m - want)[m - want]
    cand = flat[flat >= t0]
    if cand.size < kB:  # prefilter too aggressive; fall back
        cand = flat
    return np.partition(cand, cand.size - kB)[cand.size - kB]


def _rne_fp32r(a):
    """Round-to-nearest-even to 11 mantissa bits (device fp32r rounding)."""
    u = a.view(np.uint32).astype(np.uint64)
    half = np.uint64(1 << 11)
    lsb = (u >> np.uint64(12)) & np.uint64(1)
    u2 = ((u + (half - np.uint64(1)) + lsb) >> np.uint64(12)) << np.uint64(12)
    return (u2 & np.uint64(0xFFFFFFFF)).astype(np.uint32).view(np.float32)


# ------------------------------------------------------------------- kernel
def kernel(x_BZ, W_encoder_ZF, b_encoder_F, W_decoder_FZ, b_decoder_Z, k):
    k = int(np.asarray(k))
    kB = k * B
    x = np.ascontiguousarray(np.asarray(x_BZ, dtype=np.float32))
    We = np.asarray(W_encoder_ZF)
    if We.dtype != np.float32:
        We = We.astype(np.float32)
    Wd = np.ascontiguousarray(np.asarray(W_decoder_FZ, dtype=np.float32))
    be = np.ascontiguousarray(np.asarray(b_encoder_F, dtype=np.float32))
    bd = np.ascontiguousarray(np.asarray(b_decoder_Z, dtype=np.float32))

    nc_enc, nc_dec = _get_neffs()

    # ---------------- launch 1: encode
    xT = np.ascontiguousarray(x.T)
    enc_maps = []
    for c in range(NC):
        We_c = np.ascontiguousarray(We[:, c * FS:(c + 1) * FS])
        be_c = np.ascontiguousarray(be[c * FS:(c + 1) * FS].reshape(FMT, 128).T)
        enc_maps.append(dict(xT=xT, We=We_c, be=be_c))
    res1 = _run(nc_enc, enc_maps)
    actsT_sh = [r["actsT"] for r in res1]  # NC x [FS, B] float32

    # ---------------- host: exact top-(k*B) selection
    Wd64 = Wd.astype(np.float64)
    norms64 = np.sqrt(np.einsum("ij,ij->i", Wd64, Wd64))
    norms32 = norms64.astype(np.float32)

    scores = np.empty((F, B), np.float32)
    for c in range(NC):
        np.multiply(actsT_sh[c], norms32[c * FS:(c + 1) * FS, None],
                    out=scores[c * FS:(c + 1) * FS])
    flat = scores.ravel()
    vb = float(_kth_largest(flat, kB))
    hi, lo = vb + DELTA, vb - DELTA

    above = flat > hi
    n_above = int(above.sum())
    cand_idx = np.flatnonzero((flat >= lo) & (flat <= hi))
    need = kB - n_above
    if not (0 < need <= cand_idx.size):
        # window missed (shouldn't happen); widen to brute force
        cand_idx = np.flatnonzero(flat >= lo)
        need = kB - 0
        n_above = 0
        above = np.zeros_like(flat, dtype=bool)

    f_idx = (cand_idx // B).astype(np.int64)
    b_idx = (cand_idx % B).astype(np.int64)
    x64 = x.astype(np.float64)
    WeT = We.T  # [F, Z] view
    g64 = (x64[b_idx] * WeT[f_idx].astype(np.float64)).sum(axis=1)
    acts_cand64 = np.maximum(g64 + be[f_idx].astype(np.float64), 0.0)
    s_cand64 = acts_cand64 * norms64[f_idx]
    order = np.argsort(-s_cand64, kind="stable")
    sel_cand = cand_idx[order[:need]]

    # final mask = above  |  sel_cand ; lower to per-feature act thresholds
    acts_flat_dev = np.empty((F, B), np.float32)
    for c in range(NC):
        acts_flat_dev[c * FS:(c + 1) * FS] = actsT_sh[c]
    thr_f = np.full(F, HUGE, np.float32)
    above2d = above.reshape(F, B)
    anyrow = above2d.any(axis=1)
    masked = np.where(above2d[anyrow], acts_flat_dev[anyrow], HUGE)
    thr_f[anyrow] = masked.min(axis=1)
    if sel_cand.size:
        fsel = (sel_cand // B).astype(np.int64)
        vals = acts_flat_dev.ravel()[sel_cand]
        np.minimum.at(thr_f, fsel, vals)

    # device mask that thr_f realizes, and its diff vs the final mask
    dev_mask = acts_flat_dev >= thr_f[:, None]
    final_lin = np.concatenate([np.flatnonzero(above), sel_cand])
    final_mask = np.zeros(F * B, bool)
    final_mask[final_lin] = True
    final_mask = final_mask.reshape(F, B)
    diff_add = np.flatnonzero(final_mask.ravel() & ~dev_mask.ravel())   # missing on device
    diff_del = np.flatnonzero(~final_mask.ravel() & dev_mask.ravel())   # extra on device

    # ---------------- launch 2: decode
    import ml_dtypes
    Wd_bf16 = Wd.astype(ml_dtypes.bfloat16)
    zeros_bd = np.zeros((128, KT), np.float32)
    bd_arr = np.ascontiguousarray(bd.reshape(KT, 128).T)
    dec_maps = []
    for c in range(NC):
        thr_c = np.ascontiguousarray(
            thr_f[c * FS:(c + 1) * FS].reshape(FMT, 128).T)
        dec_maps.append(dict(
            actsT=actsT_sh[c],
            Wd=Wd_bf16[c * FS:(c + 1) * FS],
            thr=thr_c,
            bdec=bd_arr if c == 0 else zeros_bd,
        ))
    res2 = _run(nc_dec, dec_maps)

    # ---------------- host: assemble + patch
    sparse_full = np.concatenate(
        [r["sparseT"].astype(np.float32) for r in res2], axis=0)  # [F, B]
    rec64 = np.zeros((Z, B), np.float64)
    for r in res2:
        rec64 += r["recT"].astype(np.float64)

    sp_flat = sparse_full.ravel()
    for lin in diff_add:
        f, b = int(lin // B), int(lin % B)
        a = acts_flat_dev[f, b]
        sp_flat[lin] = a
        rec64[:, b] += float(a) * Wd64[f]
    for lin in diff_del:
        f, b = int(lin // B), int(lin % B)
        a = sp_flat[lin]
        sp_flat[lin] = 0.0
        rec64[:, b] -= float(a) * Wd64[f]

    recon = np.ascontiguousarray(rec64.T.astype(np.float32))
    return recon, sparse_full.T, acts_flat_dev.T


# revision 10
# speedup vs baseline: 1.1065x; 1.0054x over previous
"""BatchTopKCrosscoder on 8 Trainium2 NeuronCores.

Strategy (F-sharded tensor parallel, 2 device launches + thin host glue):
  Launch 1 (encode): each core computes actsT_c = relu(W_enc_c^T x^T + b)
      for its F/8 = 4096 dictionary columns, as fp32r (TF32-like) matmuls.
  Host: exact batch-top-(k*B) selection over scores = acts * ||W_dec row||.
      Bulk scores use the device acts; all scores within +-DELTA of the
      boundary are re-derived in float64 from the raw inputs so the chosen
      set matches a full-precision computation. The selection is lowered to
      per-feature activation thresholds thr_f.
  Launch 2 (decode): each core computes sparse_c = acts_c * (acts_c >= thr_c)
      and the partial reconstruction rec_c^T = W_dec_c^T sparse_c^T (+ b_dec
      on core 0 only).
  Host: sum partial reconstructions, apply O(1)-sized mask patches, assemble
      full outputs.
"""
import os
import sys
import types
import contextlib

sys.path.insert(0, "/opt/trn_rl_repo")

import numpy as np

# ---------------------------------------------------------------- constants
B, Z, F, NC = 2048, 1536, 32768, 8
FS = F // NC          # 4096 features per core
BC = 512              # matmul moving free dim
KT = Z // 128         # 12 contraction tiles (encode)
FMT = FS // 128       # 32 feature tiles per core
DELTA = 0.03          # score half-window re-derived in float64
HUGE = np.float32(3e38)

_CACHE = {}


# ------------------------------------------------------- axon profile shim
def _install_profile_shim():
    """Recreate antenv.axon_hooks (absent from the image's antenv stub) so
    run_bass_kernel_spmd(trace=True) can reach the NTFF profiler."""
    if "antenv.axon_hooks" in sys.modules:
        return
    try:
        import antenv
    except ImportError:
        return
    mod = types.ModuleType("antenv.axon_hooks")
    _hook = [None]
    mod.set_axon_ntff_profile_hook = lambda h: _hook.__setitem__(0, h)
    mod.get_axon_ntff_profile_hook = lambda: _hook[0]
    sys.modules["antenv.axon_hooks"] = mod
    antenv.axon_hooks = mod
    try:
        if "/root/.axon_site" not in sys.path:
            sys.path.append("/root/.axon_site")
        from trn_agent_boot.trn_boot import _ntff_profile_via_ctypes
        hook = _ntff_profile_via_ctypes("/opt/axon/libaxon_pjrt.so")
        if hook is not None:
            mod.set_axon_ntff_profile_hook(hook)
    except Exception:
        pass


# ------------------------------------------------------------ NEFF builders
def _env():
    if "env" in _CACHE:
        return _CACHE["env"]
    _install_profile_shim()
    from contextlib import ExitStack
    import concourse.bass as bass
    import concourse.mybir as mybir
    import concourse.tile as tile
    from concourse import bacc
    from concourse.bass_utils import run_bass_kernel_spmd
    _CACHE["env"] = (bass, mybir, tile, bacc, run_bass_kernel_spmd, ExitStack)
    return _CACHE["env"]


def _build_encode():
    bass, mybir, tile, bacc, _, ExitStack = _env()
    F32, F32R = mybir.dt.float32, mybir.dt.float32r
    nc = bacc.Bacc("TRN2", target_bir_lowering=False)
    xT = nc.dram_tensor("xT", [Z, B], F32R, kind="ExternalInput")
    We = nc.dram_tensor("We", [Z, FS], F32R, kind="ExternalInput")
    be = nc.dram_tensor("be", [128, FMT], F32, kind="ExternalInput")
    actsT = nc.dram_tensor("actsT", [FS, B], F32, kind="ExternalOutput")

    with tile.TileContext(nc) as tc, ExitStack() as ctx:
        xpool = ctx.enter_context(tc.tile_pool(name="xp", bufs=1))
        wpool = ctx.enter_context(tc.tile_pool(name="wp", bufs=2))
        opool = ctx.enter_context(tc.tile_pool(name="op", bufs=3))
        cpool = ctx.enter_context(tc.tile_pool(name="cp", bufs=1))
        pspool = ctx.enter_context(tc.tile_pool(name="pp", bufs=2, space="PSUM"))

        b_sb = cpool.tile([128, FMT], F32, name="b_sb")
        nc.sync.dma_start(b_sb[:, :], be[:, :])
        xts = []
        for zc in range(KT):
            t = xpool.tile([128, B], F32R, name=f"xt{zc}")
            nc.sync.dma_start(t[:, :], xT[zc * 128:(zc + 1) * 128, :])
            xts.append(t)

        NG = 512                       # F columns per W load group
        for g in range(FS // NG):      # 8 groups
            wg = wpool.tile([128, KT * NG], F32R, tag="wg", name=f"wg{g}")
            for zc in range(KT):
                nc.scalar.dma_start(
                    wg[:, zc * NG:(zc + 1) * NG],
                    We[zc * 128:(zc + 1) * 128, g * NG:(g + 1) * NG])
            for fm in range(NG // 128):  # 4 feature tiles per group
                ft = g * (NG // 128) + fm
                pss = [pspool.tile([128, BC], F32, tag=f"ps{b}", name=f"ps{ft}_{b}")
                       for b in range(4)]
                for zc in range(KT):
                    lhsT = wg[:, zc * NG + fm * 128: zc * NG + (fm + 1) * 128]
                    for b in range(4):
                        nc.tensor.matmul(
                            pss[b][:, :], lhsT, xts[zc][:, b * BC:(b + 1) * BC],
                            start=(zc == 0), stop=(zc == KT - 1))
                ot = opool.tile([128, B], F32, tag="ot", name=f"ot{ft}")
                for b in range(4):
                    nc.scalar.activation(
                        ot[:, b * BC:(b + 1) * BC], pss[b][:, :],
                        mybir.ActivationFunctionType.Relu, bias=b_sb[:, ft:ft + 1])
                nc.sync.dma_start(actsT[ft * 128:(ft + 1) * 128, :], ot[:, :])
    nc.compile()
    return nc


def _build_decode():
    bass, mybir, tile, bacc, _, ExitStack = _env()
    F32, BF16 = mybir.dt.float32, mybir.dt.bfloat16
    nc = bacc.Bacc("TRN2", target_bir_lowering=False)
    actsT = nc.dram_tensor("actsT", [FS, B], F32, kind="ExternalInput")
    Wd = nc.dram_tensor("Wd", [FS, Z], BF16, kind="ExternalInput")
    thr = nc.dram_tensor("thr", [128, FMT], F32, kind="ExternalInput")
    bdec = nc.dram_tensor("bdec", [128, KT], F32, kind="ExternalInput")
    sparseT = nc.dram_tensor("sparseT", [FS, B], BF16, kind="ExternalOutput")
    recT = nc.dram_tensor("recT", [Z, B], F32, kind="ExternalOutput")

    NB = 512  # moving free dim (PSUM bank limit)
    with tile.TileContext(nc) as tc, ExitStack() as ctx:
        apool = ctx.enter_context(tc.tile_pool(name="ap", bufs=5))
        sppool = ctx.enter_context(tc.tile_pool(name="sp", bufs=1))
        wpool = ctx.enter_context(tc.tile_pool(name="wp", bufs=8))
        rpool = ctx.enter_context(tc.tile_pool(name="rp", bufs=4))
        cpool = ctx.enter_context(tc.tile_pool(name="cp", bufs=1))

        thr_sb = cpool.tile([128, FMT], F32, name="thr_sb")
        nc.sync.dma_start(thr_sb[:, :], thr[:, :])
        bd_sb = cpool.tile([128, KT], F32, name="bd_sb")
        nc.sync.dma_start(bd_sb[:, :], bdec[:, :])

        # phase 1: sparse production (full batch resident in bf16, 16 MB),
        # interleaved with zg0+zg1 accumulation so the PE has work while the
        # HBM-bound acts stream arrives
        sprs = []
        with tc.tile_pool(name="pp1", bufs=1, space="PSUM") as pp1:
            ps8 = [pp1.tile([128, NB], F32, tag=f"q{i}", name=f"q{i}")
                   for i in range(8)]
            for ft in range(FMT):
                at = apool.tile([128, B], F32, tag="at", name=f"at{ft}")
                eng = nc.sync if ft % 2 == 0 else nc.scalar
                eng.dma_start(at[:, :], actsT[ft * 128:(ft + 1) * 128, :])
                sp = sppool.tile([128, B], BF16, name=f"sp{ft}")
                nc.vector.scalar_tensor_tensor(
                    sp[:, :], at[:, :], thr_sb[:, ft:ft + 1], at[:, :],
                    op0=mybir.AluOpType.is_ge, op1=mybir.AluOpType.mult)
                nc.gpsimd.dma_start(
                    sparseT[ft * 128:(ft + 1) * 128, :], sp[:, :])
                sprs.append(sp)
                for zg in (0, 1):
                    wt = wpool.tile([128, 128], BF16, tag="wt",
                                    name=f"wt{zg}_{ft}")
                    nc.scalar.dma_start(
                        wt[:, :],
                        Wd[ft * 128:(ft + 1) * 128, zg * 128:(zg + 1) * 128])
                    for s in range(4):
                        nc.tensor.matmul(
                            ps8[zg * 4 + s][:, :], wt[:, :],
                            sp[:, s * NB:(s + 1) * NB],
                            start=(ft == 0), stop=(ft == FMT - 1))
            for zg in (0, 1):
                for s in range(4):
                    rt = rpool.tile([128, NB], F32, tag="rt", name=f"rt{zg}_{s}")
                    nc.vector.tensor_scalar_add(
                        rt[:, :], ps8[zg * 4 + s][:, :], bd_sb[:, zg:zg + 1])
                    nc.gpsimd.dma_start(
                        recT[zg * 128:(zg + 1) * 128, s * NB:(s + 1) * NB],
                        rt[:, :])

        pspool = ctx.enter_context(tc.tile_pool(name="pp", bufs=2, space="PSUM"))
        for zg in range(2, KT):  # remaining z-groups; 4 psum tiles x2 bufs
            ps4 = [pspool.tile([128, NB], F32, tag=f"dp{s}", name=f"dp{zg}_{s}")
                   for s in range(4)]
            for ft in range(FMT):
                wt = wpool.tile([128, 128], BF16, tag="wt", name=f"wt{zg}_{ft}")
                nc.scalar.dma_start(
                    wt[:, :], Wd[ft * 128:(ft + 1) * 128, zg * 128:(zg + 1) * 128])
                for s in range(4):
                    nc.tensor.matmul(
                        ps4[s][:, :], wt[:, :], sprs[ft][:, s * NB:(s + 1) * NB],
                        start=(ft == 0), stop=(ft == FMT - 1))
            for s in range(4):
                rt = rpool.tile([128, NB], F32, tag="rt", name=f"rt{zg}_{s}")
                nc.vector.tensor_scalar_add(
                    rt[:, :], ps4[s][:, :], bd_sb[:, zg:zg + 1])
                nc.gpsimd.dma_start(
                    recT[zg * 128:(zg + 1) * 128, s * NB:(s + 1) * NB], rt[:, :])
    nc.compile()
    return nc


def _get_neffs():
    if "enc" not in _CACHE:
        _CACHE["enc"] = _build_encode()
        _CACHE["dec"] = _build_decode()
    return _CACHE["enc"], _CACHE["dec"]


def _run(nc, in_maps):
    _, _, _, _, run_bass_kernel_spmd, _ = _env()
    tr = os.environ.get("KERNEL_TRACE", "")
    kw = {}
    if tr:
        cores = list(range(NC)) if tr == "all" else [0]
        kw = dict(trace=True, trace_cores=cores)
    trace = bool(tr)
    try:
        res = run_bass_kernel_spmd(nc, in_maps, core_ids=list(range(NC)), **kw)
    except Exception:
        if not kw:
            raise
        # tracing occasionally races the first execute; retry untraced
        res = run_bass_kernel_spmd(nc, in_maps, core_ids=list(range(NC)))
    if kw and res.exec_time_ns is not None:
        _CACHE.setdefault("exec_times", []).append(res.exec_time_ns)
    return res.results


# ------------------------------------------------------------- host helpers
def _kth_largest(flat, kB):
    """Exact kB-th largest of a 1-D float32 array (prefilter + partition)."""
    n = flat.size
    stride = max(1, n // 200000)
    sample = flat[::stride]
    m = sample.size
    want = max(1, min(m - 1, int(kB / n * m * 1.6) + 8))
    t0 = np.partition(sample, m - want)[m - want]
    cand = flat[flat >= t0]
    if cand.size < kB:  # prefilter too aggressive; fall back
        cand = flat
    return np.partition(cand, cand.size - kB)[cand.size - kB]


def _rne_fp32r(a):
    """Round-to-nearest-even to 11 mantissa bits (device fp32r rounding)."""
    u = a.view(np.uint32).astype(np.uint64)
    half = np.uint64(1 << 11)
    lsb = (u >> np.uint64(12)) & np.uint64(1)
    u2 = ((u + (half - np.uint64(1)) + lsb) >> np.uint64(12)) << np.uint64(12)
    return (u2 & np.uint64(0xFFFFFFFF)).astype(np.uint32).view(np.float32)


# ------------------------------------------------------------------- kernel
def kernel(x_BZ, W_encoder_ZF, b_encoder_F, W_decoder_FZ, b_decoder_Z, k):
    k = int(np.asarray(k))
    kB = k * B
    x = np.ascontiguousarray(np.asarray(x_BZ, dtype=np.float32))
    We = np.asarray(W_encoder_ZF)
    if We.dtype != np.float32:
        We = We.astype(np.float32)
    Wd = np.ascontiguousarray(np.asarray(W_decoder_FZ, dtype=np.float32))
    be = np.ascontiguousarray(np.asarray(b_encoder_F, dtype=np.float32))
    bd = np.ascontiguousarray(np.asarray(b_decoder_Z, dtype=np.float32))

    nc_enc, nc_dec = _get_neffs()

    # ---------------- launch 1: encode
    xT = np.ascontiguousarray(x.T)
    enc_maps = []
    for c in range(NC):
        We_c = np.ascontiguousarray(We[:, c * FS:(c + 1) * FS])
        be_c = np.ascontiguousarray(be[c * FS:(c + 1) * FS].reshape(FMT, 128).T)
        enc_maps.append(dict(xT=xT, We=We_c, be=be_c))
    res1 = _run(nc_enc, enc_maps)
    actsT_sh = [r["actsT"] for r in res1]  # NC x [FS, B] float32

    # ---------------- host: exact top-(k*B) selection
    Wd64 = Wd.astype(np.float64)
    norms64 = np.sqrt(np.einsum("ij,ij->i", Wd64, Wd64))
    norms32 = norms64.astype(np.float32)

    scores = np.empty((F, B), np.float32)
    for c in range(NC):
        np.multiply(actsT_sh[c], norms32[c * FS:(c + 1) * FS, None],
                    out=scores[c * FS:(c + 1) * FS])
    flat = scores.ravel()
    vb = float(_kth_largest(flat, kB))
    # fp32r score error is ~1.5e-4 relative (absmax); use a 20x window,
    # scale-relative so arbitrary input scales are safe
    delta = max(3e-3 * abs(vb), 1e-6)
    for _ in range(2):
        hi, lo = vb + delta, vb - delta
        above = flat > hi
        n_above = int(above.sum())
        cand_idx = np.flatnonzero((flat >= lo) & (flat <= hi))
        need = kB - n_above
        if cand_idx.size <= 3_000_000:
            break
        delta *= 0.25
    assert 0 < need <= cand_idx.size

    f_idx = (cand_idx // B).astype(np.int64)
    b_idx = (cand_idx % B).astype(np.int64)
    x64 = x.astype(np.float64)
    WeT = We.T  # [F, Z] view
    g64 = np.empty(cand_idx.size, np.float64)
    CH = 200_000
    for i in range(0, cand_idx.size, CH):
        sl = slice(i, min(i + CH, cand_idx.size))
        g64[sl] = np.einsum(
            "ij,ij->i", x64[b_idx[sl]], WeT[f_idx[sl]].astype(np.float64))
    acts_cand64 = np.maximum(g64 + be[f_idx].astype(np.float64), 0.0)
    s_cand64 = acts_cand64 * norms64[f_idx]
    order = np.argsort(-s_cand64, kind="stable")
    sel_cand = cand_idx[order[:need]]

    # final mask = above  |  sel_cand ; lower to per-feature act thresholds
    acts_flat_dev = np.empty((F, B), np.float32)
    for c in range(NC):
        acts_flat_dev[c * FS:(c + 1) * FS] = actsT_sh[c]
    thr_f = np.full(F, HUGE, np.float32)
    above2d = above.reshape(F, B)
    anyrow = above2d.any(axis=1)
    masked = np.where(above2d[anyrow], acts_flat_dev[anyrow], HUGE)
    thr_f[anyrow] = masked.min(axis=1)
    if sel_cand.size:
        fsel = (sel_cand // B).astype(np.int64)
        vals = acts_flat_dev.ravel()[sel_cand]
        np.minimum.at(thr_f, fsel, vals)

    # device mask that thr_f realizes, and its diff vs the final mask
    dev_mask = acts_flat_dev >= thr_f[:, None]
    final_lin = np.concatenate([np.flatnonzero(above), sel_cand])
    final_mask = np.zeros(F * B, bool)
    final_mask[final_lin] = True
    final_mask = final_mask.reshape(F, B)
    diff_add = np.flatnonzero(final_mask.ravel() & ~dev_mask.ravel())   # missing on device
    diff_del = np.flatnonzero(~final_mask.ravel() & dev_mask.ravel())   # extra on device

    # ---------------- launch 2: decode
    import ml_dtypes
    Wd_bf16 = Wd.astype(ml_dtypes.bfloat16)
    zeros_bd = np.zeros((128, KT), np.float32)
    bd_arr = np.ascontiguousarray(bd.reshape(KT, 128).T)
    dec_maps = []
    for c in range(NC):
        thr_c = np.ascontiguousarray(
            thr_f[c * FS:(c + 1) * FS].reshape(FMT, 128).T)
        dec_maps.append(dict(
            actsT=actsT_sh[c],
            Wd=Wd_bf16[c * FS:(c + 1) * FS],
            thr=thr_c,
            bdec=bd_arr if c == 0 else zeros_bd,
        ))
    res2 = _run(nc_dec, dec_maps)

    # ---------------- host: assemble + patch
    sparse_full = np.concatenate(
        [r["sparseT"].astype(np.float32) for r in res2], axis=0)  # [F, B]
    rec64 = np.zeros((Z, B), np.float64)
    for r in res2:
        rec64 += r["recT"].astype(np.float64)

    sp_flat = sparse_full.ravel()
    for lin in diff_add:
        f, b = int(lin // B), int(lin % B)
        a = acts_flat_dev[f, b]
        sp_flat[lin] = a
        rec64[:, b] += float(a) * Wd64[f]
    for lin in diff_del:
        f, b = int(lin // B), int(lin % B)
        a = sp_flat[lin]
        sp_flat[lin] = 0.0
        rec64[:, b] -= float(a) * Wd64[f]

    recon = np.ascontiguousarray(rec64.T.astype(np.float32))
    return recon, sparse_full.T, acts_flat_dev.T


# revision 11
# speedup vs baseline: 1.1396x; 1.0298x over previous
"""BatchTopKCrosscoder on 8 Trainium2 NeuronCores.

Strategy (F-sharded tensor parallel, 2 device launches + thin host glue):
  Launch 1 (encode): each core computes actsT_c = relu(W_enc_c^T x^T + b)
      for its F/8 = 4096 dictionary columns, as fp32r (TF32-like) matmuls.
  Host: exact batch-top-(k*B) selection over scores = acts * ||W_dec row||.
      Bulk scores use the device acts; all scores within +-DELTA of the
      boundary are re-derived in float64 from the raw inputs so the chosen
      set matches a full-precision computation. The selection is lowered to
      per-feature activation thresholds thr_f.
  Launch 2 (decode): each core computes sparse_c = acts_c * (acts_c >= thr_c)
      and the partial reconstruction rec_c^T = W_dec_c^T sparse_c^T (+ b_dec
      on core 0 only).
  Host: sum partial reconstructions, apply O(1)-sized mask patches, assemble
      full outputs.
"""
import os
import sys
import types
import contextlib

sys.path.insert(0, "/opt/trn_rl_repo")

import numpy as np

# ---------------------------------------------------------------- constants
B, Z, F, NC = 2048, 1536, 32768, 8
FS = F // NC          # 4096 features per core
BC = 512              # matmul moving free dim
KT = Z // 128         # 12 contraction tiles (encode)
FMT = FS // 128       # 32 feature tiles per core
DELTA = 0.03          # score half-window re-derived in float64
HUGE = np.float32(3e38)

_CACHE = {}


# ------------------------------------------------------- axon profile shim
def _install_profile_shim():
    """Recreate antenv.axon_hooks (absent from the image's antenv stub) so
    run_bass_kernel_spmd(trace=True) can reach the NTFF profiler."""
    if "antenv.axon_hooks" in sys.modules:
        return
    try:
        import antenv
    except ImportError:
        return
    mod = types.ModuleType("antenv.axon_hooks")
    _hook = [None]
    mod.set_axon_ntff_profile_hook = lambda h: _hook.__setitem__(0, h)
    mod.get_axon_ntff_profile_hook = lambda: _hook[0]
    sys.modules["antenv.axon_hooks"] = mod
    antenv.axon_hooks = mod
    try:
        if "/root/.axon_site" not in sys.path:
            sys.path.append("/root/.axon_site")
        from trn_agent_boot.trn_boot import _ntff_profile_via_ctypes
        hook = _ntff_profile_via_ctypes("/opt/axon/libaxon_pjrt.so")
        if hook is not None:
            mod.set_axon_ntff_profile_hook(hook)
    except Exception:
        pass


# ------------------------------------------------------------ NEFF builders
def _env():
    if "env" in _CACHE:
        return _CACHE["env"]
    _install_profile_shim()
    from contextlib import ExitStack
    import concourse.bass as bass
    import concourse.mybir as mybir
    import concourse.tile as tile
    from concourse import bacc
    from concourse.bass_utils import run_bass_kernel_spmd
    _CACHE["env"] = (bass, mybir, tile, bacc, run_bass_kernel_spmd, ExitStack)
    return _CACHE["env"]


def _build_encode():
    bass, mybir, tile, bacc, _, ExitStack = _env()
    F32, F32R = mybir.dt.float32, mybir.dt.float32r
    nc = bacc.Bacc("TRN2", target_bir_lowering=False)
    xT = nc.dram_tensor("xT", [Z, B], F32R, kind="ExternalInput")
    We = nc.dram_tensor("We", [Z, FS], F32R, kind="ExternalInput")
    be = nc.dram_tensor("be", [128, FMT], F32, kind="ExternalInput")
    actsT = nc.dram_tensor("actsT", [FS, B], F32, kind="ExternalOutput")
    actsB = nc.dram_tensor("actsB", [FS, B], mybir.dt.bfloat16,
                           kind="ExternalOutput")

    with tile.TileContext(nc) as tc, ExitStack() as ctx:
        xpool = ctx.enter_context(tc.tile_pool(name="xp", bufs=1))
        wpool = ctx.enter_context(tc.tile_pool(name="wp", bufs=2))
        opool = ctx.enter_context(tc.tile_pool(name="op", bufs=3))
        cpool = ctx.enter_context(tc.tile_pool(name="cp", bufs=1))
        pspool = ctx.enter_context(tc.tile_pool(name="pp", bufs=2, space="PSUM"))

        b_sb = cpool.tile([128, FMT], F32, name="b_sb")
        nc.sync.dma_start(b_sb[:, :], be[:, :])
        xts = []
        for zc in range(KT):
            t = xpool.tile([128, B], F32R, name=f"xt{zc}")
            nc.sync.dma_start(t[:, :], xT[zc * 128:(zc + 1) * 128, :])
            xts.append(t)

        NG = 512                       # F columns per W load group
        for g in range(FS // NG):      # 8 groups
            wg = wpool.tile([128, KT * NG], F32R, tag="wg", name=f"wg{g}")
            for zc in range(KT):
                nc.scalar.dma_start(
                    wg[:, zc * NG:(zc + 1) * NG],
                    We[zc * 128:(zc + 1) * 128, g * NG:(g + 1) * NG])
            for fm in range(NG // 128):  # 4 feature tiles per group
                ft = g * (NG // 128) + fm
                pss = [pspool.tile([128, BC], F32, tag=f"ps{b}", name=f"ps{ft}_{b}")
                       for b in range(4)]
                for zc in range(KT):
                    lhsT = wg[:, zc * NG + fm * 128: zc * NG + (fm + 1) * 128]
                    for b in range(4):
                        nc.tensor.matmul(
                            pss[b][:, :], lhsT, xts[zc][:, b * BC:(b + 1) * BC],
                            start=(zc == 0), stop=(zc == KT - 1))
                ot = opool.tile([128, B], F32, tag="ot", name=f"ot{ft}")
                for b in range(4):
                    nc.scalar.activation(
                        ot[:, b * BC:(b + 1) * BC], pss[b][:, :],
                        mybir.ActivationFunctionType.Relu, bias=b_sb[:, ft:ft + 1])
                nc.sync.dma_start(actsT[ft * 128:(ft + 1) * 128, :], ot[:, :])
                ab = opool.tile([128, B], mybir.dt.bfloat16, tag="ab",
                                name=f"ab{ft}")
                nc.vector.tensor_copy(ab[:, :], ot[:, :])
                nc.gpsimd.dma_start(actsB[ft * 128:(ft + 1) * 128, :], ab[:, :])
    nc.compile()
    return nc


def _build_decode():
    bass, mybir, tile, bacc, _, ExitStack = _env()
    F32, BF16 = mybir.dt.float32, mybir.dt.bfloat16
    nc = bacc.Bacc("TRN2", target_bir_lowering=False)
    actsT = nc.dram_tensor("actsT", [FS, B], BF16, kind="ExternalInput")
    Wd = nc.dram_tensor("Wd", [FS, Z], BF16, kind="ExternalInput")
    thr = nc.dram_tensor("thr", [128, FMT], F32, kind="ExternalInput")
    bdec = nc.dram_tensor("bdec", [128, KT], F32, kind="ExternalInput")
    sparseT = nc.dram_tensor("sparseT", [FS, B], BF16, kind="ExternalOutput")
    recT = nc.dram_tensor("recT", [Z, B], F32, kind="ExternalOutput")

    NB = 512  # moving free dim (PSUM bank limit)
    with tile.TileContext(nc) as tc, ExitStack() as ctx:
        apool = ctx.enter_context(tc.tile_pool(name="ap", bufs=5))
        sppool = ctx.enter_context(tc.tile_pool(name="sp", bufs=1))
        wpool = ctx.enter_context(tc.tile_pool(name="wp", bufs=8))
        rpool = ctx.enter_context(tc.tile_pool(name="rp", bufs=4))
        cpool = ctx.enter_context(tc.tile_pool(name="cp", bufs=1))

        thr_sb = cpool.tile([128, FMT], F32, name="thr_sb")
        nc.sync.dma_start(thr_sb[:, :], thr[:, :])
        bd_sb = cpool.tile([128, KT], F32, name="bd_sb")
        nc.sync.dma_start(bd_sb[:, :], bdec[:, :])

        # phase 1: sparse production (full batch resident in bf16, 16 MB),
        # interleaved with zg0+zg1 accumulation so the PE has work while the
        # HBM-bound acts stream arrives
        sprs = []
        with tc.tile_pool(name="pp1", bufs=1, space="PSUM") as pp1:
            ps8 = [pp1.tile([128, NB], F32, tag=f"q{i}", name=f"q{i}")
                   for i in range(8)]
            for ft in range(FMT):
                at = apool.tile([128, B], BF16, tag="at", name=f"at{ft}")
                eng = nc.sync if ft % 2 == 0 else nc.scalar
                eng.dma_start(at[:, :], actsT[ft * 128:(ft + 1) * 128, :])
                sp = sppool.tile([128, B], BF16, name=f"sp{ft}")
                nc.vector.scalar_tensor_tensor(
                    sp[:, :], at[:, :], thr_sb[:, ft:ft + 1], at[:, :],
                    op0=mybir.AluOpType.is_ge, op1=mybir.AluOpType.mult)
                nc.gpsimd.dma_start(
                    sparseT[ft * 128:(ft + 1) * 128, :], sp[:, :])
                sprs.append(sp)
                for zg in (0, 1):
                    wt = wpool.tile([128, 128], BF16, tag="wt",
                                    name=f"wt{zg}_{ft}")
                    nc.scalar.dma_start(
                        wt[:, :],
                        Wd[ft * 128:(ft + 1) * 128, zg * 128:(zg + 1) * 128])
                    for s in range(4):
                        nc.tensor.matmul(
                            ps8[zg * 4 + s][:, :], wt[:, :],
                            sp[:, s * NB:(s + 1) * NB],
                            start=(ft == 0), stop=(ft == FMT - 1))
            for zg in (0, 1):
                for s in range(4):
                    rt = rpool.tile([128, NB], F32, tag="rt", name=f"rt{zg}_{s}")
                    nc.vector.tensor_scalar_add(
                        rt[:, :], ps8[zg * 4 + s][:, :], bd_sb[:, zg:zg + 1])
                    nc.gpsimd.dma_start(
                        recT[zg * 128:(zg + 1) * 128, s * NB:(s + 1) * NB],
                        rt[:, :])

        pspool = ctx.enter_context(tc.tile_pool(name="pp", bufs=2, space="PSUM"))
        for zg in range(2, KT):  # remaining z-groups; 4 psum tiles x2 bufs
            ps4 = [pspool.tile([128, NB], F32, tag=f"dp{s}", name=f"dp{zg}_{s}")
                   for s in range(4)]
            for ft in range(FMT):
                wt = wpool.tile([128, 128], BF16, tag="wt", name=f"wt{zg}_{ft}")
                nc.scalar.dma_start(
                    wt[:, :], Wd[ft * 128:(ft + 1) * 128, zg * 128:(zg + 1) * 128])
                for s in range(4):
                    nc.tensor.matmul(
                        ps4[s][:, :], wt[:, :], sprs[ft][:, s * NB:(s + 1) * NB],
                        start=(ft == 0), stop=(ft == FMT - 1))
            for s in range(4):
                rt = rpool.tile([128, NB], F32, tag="rt", name=f"rt{zg}_{s}")
                nc.vector.tensor_scalar_add(
                    rt[:, :], ps4[s][:, :], bd_sb[:, zg:zg + 1])
                nc.gpsimd.dma_start(
                    recT[zg * 128:(zg + 1) * 128, s * NB:(s + 1) * NB], rt[:, :])
    nc.compile()
    return nc


def _get_neffs():
    if "enc" not in _CACHE:
        _CACHE["enc"] = _build_encode()
        _CACHE["dec"] = _build_decode()
    return _CACHE["enc"], _CACHE["dec"]


def _run(nc, in_maps):
    _, _, _, _, run_bass_kernel_spmd, _ = _env()
    tr = os.environ.get("KERNEL_TRACE", "")
    kw = {}
    if tr:
        cores = list(range(NC)) if tr == "all" else [0]
        kw = dict(trace=True, trace_cores=cores)
    trace = bool(tr)
    try:
        res = run_bass_kernel_spmd(nc, in_maps, core_ids=list(range(NC)), **kw)
    except Exception:
        if not kw:
            raise
        # tracing occasionally races the first execute; retry untraced
        res = run_bass_kernel_spmd(nc, in_maps, core_ids=list(range(NC)))
    if kw and res.exec_time_ns is not None:
        _CACHE.setdefault("exec_times", []).append(res.exec_time_ns)
    return res.results


# ------------------------------------------------------------- host helpers
def _kth_largest(flat, kB):
    """Exact kB-th largest of a 1-D float32 array (prefilter + partition)."""
    n = flat.size
    stride = max(1, n // 200000)
    sample = flat[::stride]
    m = sample.size
    want = max(1, min(m - 1, int(kB / n * m * 1.6) + 8))
    t0 = np.partition(sample, m - want)[m - want]
    cand = flat[flat >= t0]
    if cand.size < kB:  # prefilter too aggressive; fall back
        cand = flat
    return np.partition(cand, cand.size - kB)[cand.size - kB]


def _rne_fp32r(a):
    """Round-to-nearest-even to 11 mantissa bits (device fp32r rounding)."""
    u = a.view(np.uint32).astype(np.uint64)
    half = np.uint64(1 << 11)
    lsb = (u >> np.uint64(12)) & np.uint64(1)
    u2 = ((u + (half - np.uint64(1)) + lsb) >> np.uint64(12)) << np.uint64(12)
    return (u2 & np.uint64(0xFFFFFFFF)).astype(np.uint32).view(np.float32)


# ------------------------------------------------------------------- kernel
def kernel(x_BZ, W_encoder_ZF, b_encoder_F, W_decoder_FZ, b_decoder_Z, k):
    k = int(np.asarray(k))
    kB = k * B
    x = np.ascontiguousarray(np.asarray(x_BZ, dtype=np.float32))
    We = np.asarray(W_encoder_ZF)
    if We.dtype != np.float32:
        We = We.astype(np.float32)
    Wd = np.ascontiguousarray(np.asarray(W_decoder_FZ, dtype=np.float32))
    be = np.ascontiguousarray(np.asarray(b_encoder_F, dtype=np.float32))
    bd = np.ascontiguousarray(np.asarray(b_decoder_Z, dtype=np.float32))

    nc_enc, nc_dec = _get_neffs()

    # ---------------- launch 1: encode
    xT = np.ascontiguousarray(x.T)
    enc_maps = []
    for c in range(NC):
        We_c = np.ascontiguousarray(We[:, c * FS:(c + 1) * FS])
        be_c = np.ascontiguousarray(be[c * FS:(c + 1) * FS].reshape(FMT, 128).T)
        enc_maps.append(dict(xT=xT, We=We_c, be=be_c))
    res1 = _run(nc_enc, enc_maps)
    actsT_sh = [r["actsT"] for r in res1]  # NC x [FS, B] float32
    actsB_sh = [r["actsB"] for r in res1]  # NC x [FS, B] bfloat16

    # ---------------- host: exact top-(k*B) selection
    Wd64 = Wd.astype(np.float64)
    norms64 = np.sqrt(np.einsum("ij,ij->i", Wd64, Wd64))
    norms32 = norms64.astype(np.float32)

    scores = np.empty((F, B), np.float32)
    for c in range(NC):
        np.multiply(actsT_sh[c], norms32[c * FS:(c + 1) * FS, None],
                    out=scores[c * FS:(c + 1) * FS])
    flat = scores.ravel()
    vb = float(_kth_largest(flat, kB))
    # fp32r score error is ~1.5e-4 relative (absmax); use a 20x window,
    # scale-relative so arbitrary input scales are safe
    delta = max(3e-3 * abs(vb), 1e-6)
    for _ in range(2):
        hi, lo = vb + delta, vb - delta
        above = flat > hi
        n_above = int(above.sum())
        cand_idx = np.flatnonzero((flat >= lo) & (flat <= hi))
        need = kB - n_above
        if cand_idx.size <= 3_000_000:
            break
        delta *= 0.25
    assert 0 < need <= cand_idx.size

    f_idx = (cand_idx // B).astype(np.int64)
    b_idx = (cand_idx % B).astype(np.int64)
    x64 = x.astype(np.float64)
    WeT = We.T  # [F, Z] view
    g64 = np.empty(cand_idx.size, np.float64)
    CH = 200_000
    for i in range(0, cand_idx.size, CH):
        sl = slice(i, min(i + CH, cand_idx.size))
        g64[sl] = np.einsum(
            "ij,ij->i", x64[b_idx[sl]], WeT[f_idx[sl]].astype(np.float64))
    acts_cand64 = np.maximum(g64 + be[f_idx].astype(np.float64), 0.0)
    s_cand64 = acts_cand64 * norms64[f_idx]
    order = np.argsort(-s_cand64, kind="stable")
    sel_cand = cand_idx[order[:need]]

    # final mask = above  |  sel_cand ; lower to per-feature act thresholds
    # in the bf16-acts domain the decode kernel sees (thresholds are exact
    # bf16-representable values so host/device predicates agree bitwise)
    acts_flat_dev = np.empty((F, B), np.float32)
    for c in range(NC):
        acts_flat_dev[c * FS:(c + 1) * FS] = actsT_sh[c]
    ab32 = np.empty((F, B), np.float32)
    for c in range(NC):
        ab32[c * FS:(c + 1) * FS] = actsB_sh[c].astype(np.float32)
    thr_f = np.full(F, HUGE, np.float32)
    above2d = above.reshape(F, B)
    anyrow = above2d.any(axis=1)
    masked = np.where(above2d[anyrow], ab32[anyrow], HUGE)
    thr_f[anyrow] = masked.min(axis=1)
    if sel_cand.size:
        fsel = (sel_cand // B).astype(np.int64)
        vals = ab32.ravel()[sel_cand]
        np.minimum.at(thr_f, fsel, vals)

    # device mask that thr_f realizes, and its diff vs the final mask
    dev_mask = ab32 >= thr_f[:, None]
    final_lin = np.concatenate([np.flatnonzero(above), sel_cand])
    final_mask = np.zeros(F * B, bool)
    final_mask[final_lin] = True
    final_mask = final_mask.reshape(F, B)
    diff_add = np.flatnonzero(final_mask.ravel() & ~dev_mask.ravel())   # missing on device
    diff_del = np.flatnonzero(~final_mask.ravel() & dev_mask.ravel())   # extra on device

    # ---------------- launch 2: decode
    import ml_dtypes
    Wd_bf16 = Wd.astype(ml_dtypes.bfloat16)
    zeros_bd = np.zeros((128, KT), np.float32)
    bd_arr = np.ascontiguousarray(bd.reshape(KT, 128).T)
    dec_maps = []
    for c in range(NC):
        thr_c = np.ascontiguousarray(
            thr_f[c * FS:(c + 1) * FS].reshape(FMT, 128).T)
        dec_maps.append(dict(
            actsT=actsB_sh[c],
            Wd=Wd_bf16[c * FS:(c + 1) * FS],
            thr=thr_c,
            bdec=bd_arr if c == 0 else zeros_bd,
        ))
    res2 = _run(nc_dec, dec_maps)

    # ---------------- host: assemble + patch
    sparse_full = np.concatenate(
        [r["sparseT"].astype(np.float32) for r in res2], axis=0)  # [F, B]
    rec64 = np.zeros((Z, B), np.float64)
    for r in res2:
        rec64 += r["recT"].astype(np.float64)

    sp_flat = sparse_full.ravel()
    for lin in diff_add:
        f, b = int(lin // B), int(lin % B)
        a = ab32[f, b]
        sp_flat[lin] = a
        rec64[:, b] += float(a) * Wd64[f]
    for lin in diff_del:
        f, b = int(lin // B), int(lin % B)
        a = sp_flat[lin]
        sp_flat[lin] = 0.0
        rec64[:, b] -= float(a) * Wd64[f]

    recon = np.ascontiguousarray(rec64.T.astype(np.float32))
    return recon, sparse_full.T, acts_flat_dev.T
